# revision 8
# baseline (speedup 1.0000x reference)
"""Trainium2 Bass kernel for BaselineProtonet (retrieval_knn).

logits[q, c] = -||query_q - proto_c||_2
  proto_c = mean of 64 support embeddings of class c
  embeddings_stacked: [64 classes * (64 support + 64 query), 1024] f32

Sharding (8 cores, 2D grid): 4 query blocks x 2 class blocks. Core
(i, j) owns query rows 1024*i..1024*(i+1) and classes 32*j..32*(j+1).
Per-core wire traffic is 2MB support + 1MB queries (both fp8) vs 5.6MB
for the query-only sharding -- DMA is the critical path at ~333GB/s.

Host-side shard prep (layout/encoding only, no arithmetic): support is
swizzled d-half-major with partition p owning class p//4 in every
256-row DoubleRow chunk (one one-hot weight load serves all chunks);
queries go feature-major; both encode fp8e4m3.

The DMA ring interleaves the two tensors so every serial chain hides
under later stream phases:
  qryA (d-half0) | supH0 (2 slices) | qryB (d-half1) | supH1 (2 slices)
  - q*q squares start early, spread over DVE + GPSIMD (fp8 elementwise
    is slow on both: ~1.2-2.9us per [128,1024] chunk)
  - prototype bank h0 closes mid-kernel: ACT evac (1/64), PE transposes,
    ACT *-2 -> W8 fp8, then gram matmuls for d-half0 run during supH1
  - only the h1 W-chain + 2 gram pairs + ||q||^2 stragglers trail the
    last DMA byte, then sqrt(+||p||^2 bias) -> negate -> f32 out
"""

import numpy as np

C = 64          # classes
S = 64          # support per class (== queries per class)
D = 1024        # embedding dim
NCORES = 8
QB = 4          # query blocks
CB = 2          # class blocks
CL = C // CB    # 32 classes per core
QL = (C * S) // QB          # 1024 query rows per core
SJP = (CL * S) // 256       # 8 support chunk-pairs per core
KP = D // 256               # 4 feature chunk-pairs

_CACHE = {}


def _emit(nc, tc, sup, qt, oh_in, out):
    """Emit the per-core tile program.

    sup:   [128, 2*SJP*2*512] fp8 DRAM (support, [h, jp, o, d512] cols,
                                        partition p owns class p//4)
    qt:    [128, KP*2*QL] fp8 DRAM     (queries, feature-major)
    oh_in: [128, 2*CL] fp8 DRAM        (DoubleRow one-hot class mask)
    out:   [CL, QL] f32 DRAM           (negated distances, class-major)
    """
    from concourse import masks, mybir

    f32 = mybir.dt.float32
    bf16 = mybir.dt.bfloat16
    fp8 = mybir.dt.float8e4
    AF = mybir.ActivationFunctionType
    DR = mybir.MatmulPerfMode.DoubleRow

    with (
        tc.tile_pool(name="sb", bufs=1) as sb,
        tc.tile_pool(name="ps", bufs=1, space="PSUM") as ps,
    ):
        # warm the PE clock first-thing: HAM un-throttles only after
        # ~3.4us of sustained matmul activity, so burn ~4.3us of dummy
        # matmuls while the input DMAs stream
        wm_in = sb.tile([128, 512], bf16)
        nc.gpsimd.memset(wm_in[:], 0.0)
        wm_ps = ps.tile([128, 512], f32)
        for _ in range(11):
            nc.tensor.matmul(
                wm_ps[:], wm_in[:, 0:128], wm_in[:], start=True, stop=True
            )

        # ---------------- input DMA ring (one sync-queue FIFO) ----------
        oh = sb.tile([128, 2, CL], fp8)
        nc.scalar.dma_start(
            oh[:], oh_in[:, :].rearrange("p (o c) -> p o c", o=2)
        )
        q8 = sb.tile([128, KP, 2, QL], fp8)
        s8 = sb.tile([128, 2, SJP, 2, 512], fp8)

        def q_slice(g):  # 512KB: d-chunk-pairs 2g, 2g+1, all queries
            nc.sync.dma_start(
                q8[:, 2 * g : 2 * (g + 1)],
                qt[:, 4096 * g : 4096 * (g + 1)].rearrange(
                    "p (kp o q) -> p kp o q", kp=2, o=2
                ),
            )

        def s_slice(h, jp0, njp):  # d-half h, chunk-pairs jp0..jp0+njp
            nc.sync.dma_start(
                s8[:, h, jp0 : jp0 + njp],
                sup[
                    :, 8192 * h + 1024 * jp0 : 8192 * h + 1024 * (jp0 + njp)
                ].rearrange("p (jp o d) -> p jp o d", jp=njp, o=2),
            )

        # all slices 512KB = 4KB per partition line: the SDMA engines are
        # descriptor-latency bound (~155ns per line), so 2KB-line slices
        # stream at ~220GB/s and 1KB-line at ~110 vs ~340 for 4KB
        q_slice(0)          # qryA: d-half0
        s_slice(0, 0, 4)
        s_slice(0, 4, 4)
        q_slice(1)          # qryB: d-half1
        s_slice(1, 0, 4)
        s_slice(1, 4, 4)

        # ---------------- constants -------------------------------------
        ident = sb.tile([128, 128], bf16)
        masks.make_identity(nc, ident[:])
        on8 = sb.tile([128, 2, CL], fp8)
        nc.gpsimd.memset(on8[:], 1.0)

        # preload the sqrt ACT table off the critical path
        warm_sq = sb.tile([1, 1], f32)
        nc.gpsimd.memset(warm_sq[:], 1.0)
        nc.scalar.activation(warm_sq[:], warm_sq[:], AF.Sqrt)

        # ---------------- q*q squares (early, DVE + GPSIMD) -------------
        # chunk k = (kp, o); DVE: 0,1,4,5,7  GPSIMD: 2,3,6 (GPSIMD is a
        # constant ~2.9us per chunk; DVE is 1.2us alone, up to 2.9us
        # under contention, so the late chunks lean on both)
        qsq = sb.tile([128, KP, 2, QL], fp8)

        def sq(k, eng):
            kp, o = divmod(k, 2)
            eng.tensor_mul(qsq[:, kp, o], q8[:, kp, o], q8[:, kp, o])

        sq(0, nc.vector)
        sq(1, nc.vector)
        sq(2, nc.gpsimd)
        sq(3, nc.gpsimd)

        # dummy matmuls (no deps, low priority): the PE scheduler runs
        # them only when nothing real is ready, keeping the HAM clock
        # gate warm through DMA/evac waits
        def dummy_mm(n=1):
            for _ in range(n):
                nc.tensor.matmul(
                    wm_ps[:], wm_in[:, 0:128], wm_in[:], start=True, stop=True
                )

        # ---------------- prototypes: d-half 0 --------------------------
        p_ps = ps.tile([CL, D], f32)  # [32, 1024] = 2 banks
        for jp in range(SJP):
            nc.tensor.matmul(
                p_ps[:, 0:512],
                oh[:],
                s8[:, 0, jp],
                start=(jp == 0),
                stop=(jp == SJP - 1),
                perf_mode=DR,
            )

        dummy_mm(2)

        # W chain A: evac protos h0, transpose, scale to fp8; quarter
        # [32,256] evacs let the transposes and W scales pipeline
        psbA = sb.tile([CL, 512], bf16)
        pt_ps = ps.tile([128, 2 * KP, CL], bf16)  # [128, 8*32] = 1 bank
        W8 = sb.tile([128, KP, 2, CL], fp8)

        def w_chain(h, psb):
            src_ps = p_ps[:, 512 * h : 512 * (h + 1)]
            for u in range(2):  # kp = 2h+u
                us = slice(256 * u, 256 * (u + 1))
                nc.scalar.mul(psb[:, us], src_ps[:, us], 1.0 / S)
                for kk in range(2):
                    k = 4 * h + 2 * u + kk
                    nc.tensor.transpose(
                        pt_ps[:, k],
                        psb[:, 256 * u + 128 * kk : 256 * u + 128 * (kk + 1)],
                        ident[0:CL, 0:CL],
                    )
                kp = 2 * h + u
                nc.scalar.mul(
                    W8[:, kp].rearrange("p o c -> p (o c)"),
                    pt_ps[:, 2 * kp : 2 * kp + 2].rearrange("p k c -> p (k c)"),
                    -2.0,
                )

        w_chain(0, psbA)
        # ||p||^2 half A on ACT (idle window), accumulate along d
        sq_dumpA = sb.tile([CL, 512], bf16)
        pnA = sb.tile([CL, 1], f32)
        nc.scalar.activation(sq_dumpA[:], psbA[:], AF.Square, accum_out=pnA[:])

        # gram for d-half0 runs during the supH1 stream
        s_ps = ps.tile([CL, QL], f32)  # [32, 1024] = 2 banks
        for kp in range(2):
            for qh in range(2):
                qs = slice(512 * qh, 512 * (qh + 1))
                nc.tensor.matmul(
                    s_ps[:, qs],
                    W8[:, kp],
                    q8[:, kp, :, qs],
                    start=(kp == 0),
                    stop=False,
                    perf_mode=DR,
                )
        sq(4, nc.vector)
        sq(7, nc.vector)
        sq(6, nc.gpsimd)
        sq(5, nc.vector)
        dummy_mm(2)

        # ---------------- prototypes: d-half 1 --------------------------
        for jp in range(SJP):
            nc.tensor.matmul(
                p_ps[:, 512:1024],
                oh[:],
                s8[:, 1, jp],
                start=(jp == 0),
                stop=(jp == SJP - 1),
                perf_mode=DR,
            )

        # ||q||^2 chunk-pairs 0, 1 while the W chain B evacuates
        for kp in (0, 1):
            for qh in range(2):
                qs = slice(512 * qh, 512 * (qh + 1))
                nc.tensor.matmul(
                    s_ps[:, qs], on8[:], qsq[:, kp, :, qs],
                    start=False, stop=False, perf_mode=DR,
                )

        # W chain B
        psbB = sb.tile([CL, 512], bf16)
        w_chain(1, psbB)

        # tail matmuls: ||q||^2 pair 2 first (no W dependency), then the
        # half-1 gram, then pair 3 closes the banks
        for qh in range(2):
            qs = slice(512 * qh, 512 * (qh + 1))
            nc.tensor.matmul(
                s_ps[:, qs], on8[:], qsq[:, 2, :, qs],
                start=False, stop=False, perf_mode=DR,
            )
        for kp in range(2, 4):
            for qh in range(2):
                qs = slice(512 * qh, 512 * (qh + 1))
                nc.tensor.matmul(
                    s_ps[:, qs],
                    W8[:, kp],
                    q8[:, kp, :, qs],
                    start=False,
                    stop=False,
                    perf_mode=DR,
                )
        for qh in range(2):
            qs = slice(512 * qh, 512 * (qh + 1))
            nc.tensor.matmul(
                s_ps[:, qs], on8[:], qsq[:, 3, :, qs],
                start=False, stop=True, perf_mode=DR,
            )

        # ||p||^2 half B + total
        sq_dumpB = sb.tile([CL, 512], bf16)
        pnB = sb.tile([CL, 1], f32)
        pn = sb.tile([CL, 1], f32)
        nc.scalar.activation(sq_dumpB[:], psbB[:], AF.Square, accum_out=pnB[:])
        nc.vector.tensor_add(pn[:], pnA[:], pnB[:])

        # ------- sqrt(+||p||^2), negate, store (2 q-halves pipelined) ---
        for qh in range(2):
            qs = slice(512 * qh, 512 * (qh + 1))
            lt = sb.tile([CL, 512], f32, name=f"lt{qh}")
            ltn = sb.tile([CL, 512], f32, name=f"ltn{qh}")
            nc.scalar.activation(lt[:], s_ps[:, qs], AF.Sqrt, bias=pn[:, 0:1])
            nc.vector.tensor_scalar_mul(ltn[:], lt[:], -1.0)
            nc.sync.dma_start(out[:, qs], ltn[:])



def _build():
    if "nc" in _CACHE:
        return _CACHE["nc"]
    from concourse import bacc, mybir, tile

    f32 = mybir.dt.float32
    fp8 = mybir.dt.float8e4
    nc = bacc.Bacc(
        "TRN2",
        target_bir_lowering=False,
        debug=False,
        enable_asserts=False,
        num_devices=NCORES,
    )
    sup = nc.dram_tensor("sup", [128, 2 * SJP * 2 * 512], fp8, kind="ExternalInput").ap()
    qt = nc.dram_tensor("qt", [128, KP * 2 * QL], fp8, kind="ExternalInput").ap()
    oh_in = nc.dram_tensor("oh", [128, 2 * CL], fp8, kind="ExternalInput").ap()
    out = nc.dram_tensor("out", [CL, QL], f32, kind="ExternalOutput").ap()
    with tile.TileContext(nc) as tc:
        _emit(nc, tc, sup, qt, oh_in, out)
    nc.compile()
    _CACHE["nc"] = nc
    return nc


def _shard(embeddings):
    import ml_dtypes

    emb = np.asarray(embeddings, dtype=np.float32).reshape(C, 2 * S, D)
    support = emb[:, :S, :]                       # [64, 64, 1024]
    queries = emb[:, S:, :].reshape(C * S, D)     # [4096, 1024]

    # one-hot: oh[p, o, c] = 1 iff c == p//4 (same for every chunk-pair)
    p = np.arange(128)[:, None, None]
    c = np.arange(CL)[None, None, :]
    oh = (c == p // 4) + np.zeros((1, 2, 1), dtype=bool)
    oh = np.ascontiguousarray(
        oh.astype(ml_dtypes.float8_e4m3).reshape(128, 2 * CL)
    )

    sups = []
    for j in range(CB):
        # [32, 64, 1024] -> [c, jp, o, m, h, 512] -> [c, m, h, jp, o, 512]
        sj = support[CL * j : CL * (j + 1)].reshape(CL, SJP, 2, 4, 2, 512)
        sj = sj.transpose(0, 3, 4, 1, 2, 5).reshape(128, 2 * SJP * 2 * 512)
        sups.append(np.ascontiguousarray(sj.astype(ml_dtypes.float8_e4m3)))
    qts = []
    for i in range(QB):
        # [1024q, 1024d] -> T -> [4kp, 2o, 128p, 1024q] -> [p, kp, o, q]
        qi = queries[QL * i : QL * (i + 1)].T.reshape(KP, 2, 128, QL)
        qi = qi.transpose(2, 0, 1, 3).reshape(128, KP * 2 * QL)
        qts.append(np.ascontiguousarray(qi.astype(ml_dtypes.float8_e4m3)))

    in_maps = []
    for k in range(NCORES):
        i, j = k // CB, k % CB
        in_maps.append({"sup": sups[j], "qt": qts[i], "oh": oh})
    return in_maps


def _gather(outs):
    """outs: list of 8 per-core [CL, QL] arrays -> full [C*S, C] f32."""
    logits = np.empty((C * S, C), dtype=np.float32)
    for k in range(NCORES):
        i, j = k // CB, k % CB
        logits[QL * i : QL * (i + 1), CL * j : CL * (j + 1)] = np.asarray(
            outs[k], dtype=np.float32
        ).T
    return logits


def kernel(embeddings_stacked, n_classes, n_support, **_unused):
    assert int(n_classes) == C and int(n_support) == S
    emb = np.asarray(embeddings_stacked)
    assert emb.shape == (C * 2 * S, D), emb.shape

    from concourse import bass_utils

    nc = _build()
    in_maps = _shard(emb)
    try:
        res = bass_utils.run_bass_kernel_spmd(
            nc, in_maps, core_ids=list(range(NCORES))
        )
    except Exception:
        # transient device/runtime hiccups have been observed; retry once
        res = bass_utils.run_bass_kernel_spmd(
            nc, in_maps, core_ids=list(range(NCORES))
        )
    return _gather([res.results[k]["out"] for k in range(NCORES)])


if __name__ == "__main__":
    rng = np.random.default_rng(0)
    emb = rng.standard_normal((C * 2 * S, D), dtype=np.float32)
    got = kernel(emb, C, S)
    print("kernel output", got.shape, got.dtype)


# revision 9
# speedup vs baseline: 1.0292x; 1.0292x over previous
"""Trainium2 Bass kernel for BaselineProtonet (retrieval_knn).

logits[q, c] = -||query_q - proto_c||_2
  proto_c = mean of 64 support embeddings of class c
  embeddings_stacked: [64 classes * (64 support + 64 query), 1024] f32

Sharding (8 cores, 2D grid): 4 query blocks x 2 class blocks. Core
(i, j) owns query rows 1024*i..1024*(i+1) and classes 32*j..32*(j+1).
Per-core wire traffic is 2MB support + 1MB queries (both fp8) vs 5.6MB
for the query-only sharding -- DMA is the critical path at ~333GB/s.

Host-side shard prep (layout/encoding only, no arithmetic): support is
swizzled d-half-major with partition p owning class p//4 in every
256-row DoubleRow chunk (one one-hot weight load serves all chunks);
queries go feature-major; both encode fp8e4m3.

The DMA ring interleaves the two tensors so every serial chain hides
under later stream phases:
  qryA (d-half0) | supH0 (2 slices) | qryB (d-half1) | supH1 (2 slices)
  - q*q squares start early, spread over DVE + GPSIMD (fp8 elementwise
    is slow on both: ~1.2-2.9us per [128,1024] chunk)
  - prototype bank h0 closes mid-kernel: ACT evac (1/64), PE transposes,
    ACT *-2 -> W8 fp8, then gram matmuls for d-half0 run during supH1
  - only the h1 W-chain + 2 gram pairs + ||q||^2 stragglers trail the
    last DMA byte, then sqrt(+||p||^2 bias) -> negate -> f32 out
"""

import numpy as np

C = 64          # classes
S = 64          # support per class (== queries per class)
D = 1024        # embedding dim
NCORES = 8
QB = 4          # query blocks
CB = 2          # class blocks
CL = C // CB    # 32 classes per core
QL = (C * S) // QB          # 1024 query rows per core
SJP = (CL * S) // 256       # 8 support chunk-pairs per core
KP = D // 256               # 4 feature chunk-pairs

_CACHE = {}


def _emit(nc, tc, sup, qt, oh_in, out):
    """Emit the per-core tile program.

    sup:   [128, 2*SJP*2*512] fp8 DRAM (support, [h, jp, o, d512] cols,
                                        partition p owns class p//4)
    qt:    [128, KP*2*QL] fp8 DRAM     (queries, feature-major)
    oh_in: [128, 2*CL] fp8 DRAM        (DoubleRow one-hot class mask)
    out:   [CL, QL] f32 DRAM           (negated distances, class-major)
    """
    from concourse import masks, mybir

    f32 = mybir.dt.float32
    bf16 = mybir.dt.bfloat16
    fp8 = mybir.dt.float8e4
    AF = mybir.ActivationFunctionType
    DR = mybir.MatmulPerfMode.DoubleRow

    with (
        tc.tile_pool(name="sb", bufs=1) as sb,
        tc.tile_pool(name="ps", bufs=1, space="PSUM") as ps,
    ):
        # warm the PE clock first-thing: HAM un-throttles only after
        # ~3.4us of sustained matmul activity, so burn ~4.3us of dummy
        # matmuls while the input DMAs stream
        wm_in = sb.tile([128, 512], bf16)
        nc.gpsimd.memset(wm_in[:], 0.0)
        wm_ps = ps.tile([128, 512], f32)
        for _ in range(8):
            nc.tensor.matmul(
                wm_ps[:], wm_in[:, 0:128], wm_in[:], start=True, stop=True
            )

        # ---------------- input DMA ring (one sync-queue FIFO) ----------
        oh = sb.tile([128, 2, CL], fp8)
        nc.scalar.dma_start(
            oh[:], oh_in[:, :].rearrange("p (o c) -> p o c", o=2)
        )
        q8 = sb.tile([128, KP, 2, QL], fp8)
        s8 = sb.tile([128, 2, SJP, 2, 512], fp8)

        def q_slice(g):  # 512KB: d-chunk-pairs 2g, 2g+1, all queries
            nc.sync.dma_start(
                q8[:, 2 * g : 2 * (g + 1)],
                qt[:, 4096 * g : 4096 * (g + 1)].rearrange(
                    "p (kp o q) -> p kp o q", kp=2, o=2
                ),
            )

        def s_slice(h, jp0, njp):  # d-half h, chunk-pairs jp0..jp0+njp
            nc.sync.dma_start(
                s8[:, h, jp0 : jp0 + njp],
                sup[
                    :, 8192 * h + 1024 * jp0 : 8192 * h + 1024 * (jp0 + njp)
                ].rearrange("p (jp o d) -> p jp o d", jp=njp, o=2),
            )

        # all slices 512KB = 4KB per partition line: the SDMA engines are
        # descriptor-latency bound (~155ns per line), so 2KB-line slices
        # stream at ~220GB/s and 1KB-line at ~110 vs ~340 for 4KB
        s_slice(0, 0, 4)    # supH0a: protos start right after warmup
        q_slice(0)          # qryA: d-half0 queries, feeds the squares
        s_slice(0, 4, 4)    # supH0b
        s_slice(1, 0, 4)    # supH1a
        q_slice(1)          # qryB: d-half1 queries
        s_slice(1, 4, 4)    # supH1b: gates the whole B tail chain

        # ---------------- constants -------------------------------------
        ident = sb.tile([128, 128], bf16)
        masks.make_identity(nc, ident[:])
        on8 = sb.tile([128, 2, CL], fp8)
        nc.gpsimd.memset(on8[:], 1.0)

        # preload the sqrt ACT table off the critical path
        warm_sq = sb.tile([1, 1], f32)
        nc.gpsimd.memset(warm_sq[:], 1.0)
        nc.scalar.activation(warm_sq[:], warm_sq[:], AF.Sqrt)

        # ---------------- q*q squares (early, DVE + GPSIMD) -------------
        # chunk k = (kp, o); DVE: 0,1,4,5,7  GPSIMD: 2,3,6 (GPSIMD is a
        # constant ~2.9us per chunk; DVE is 1.2us alone, up to 2.9us
        # under contention, so the late chunks lean on both)
        qsq = sb.tile([128, KP, 2, QL], fp8)

        def sq(k, eng):
            kp, o = divmod(k, 2)
            eng.tensor_mul(qsq[:, kp, o], q8[:, kp, o], q8[:, kp, o])

        sq(0, nc.vector)
        sq(1, nc.vector)
        sq(2, nc.gpsimd)
        sq(3, nc.gpsimd)

        # dummy matmuls (no deps, low priority): the PE scheduler runs
        # them only when nothing real is ready, keeping the HAM clock
        # gate warm through DMA/evac waits
        def dummy_mm(n=1):
            for _ in range(n):
                nc.tensor.matmul(
                    wm_ps[:], wm_in[:, 0:128], wm_in[:], start=True, stop=True
                )

        # ---------------- prototypes: d-half 0 --------------------------
        p_ps = ps.tile([CL, D], f32)  # [32, 1024] = 2 banks
        for jp in range(SJP):
            nc.tensor.matmul(
                p_ps[:, 0:512],
                oh[:],
                s8[:, 0, jp],
                start=(jp == 0),
                stop=(jp == SJP - 1),
                perf_mode=DR,
            )

        dummy_mm(4)

        # W chain A: evac protos h0, transpose, scale to fp8; quarter
        # [32,256] evacs let the transposes and W scales pipeline
        psbA = sb.tile([CL, 512], bf16)
        pt_ps = ps.tile([128, 2 * KP, CL], bf16)  # [128, 8*32] = 1 bank
        W8 = sb.tile([128, KP, 2, CL], fp8)

        def w_chain(h, psb):
            src_ps = p_ps[:, 512 * h : 512 * (h + 1)]
            for u in range(2):  # kp = 2h+u
                us = slice(256 * u, 256 * (u + 1))
                nc.scalar.mul(psb[:, us], src_ps[:, us], 1.0 / S)
                for kk in range(2):
                    k = 4 * h + 2 * u + kk
                    nc.tensor.transpose(
                        pt_ps[:, k],
                        psb[:, 256 * u + 128 * kk : 256 * u + 128 * (kk + 1)],
                        ident[0:CL, 0:CL],
                    )
                kp = 2 * h + u
                nc.scalar.mul(
                    W8[:, kp].rearrange("p o c -> p (o c)"),
                    pt_ps[:, 2 * kp : 2 * kp + 2].rearrange("p k c -> p (k c)"),
                    -2.0,
                )

        w_chain(0, psbA)
        # ||p||^2 half A on ACT (idle window), accumulate along d
        sq_dumpA = sb.tile([CL, 512], bf16)
        pnA = sb.tile([CL, 1], f32)
        nc.scalar.activation(sq_dumpA[:], psbA[:], AF.Square, accum_out=pnA[:])

        # gram for d-half0 runs during the supH1 stream
        s_ps = ps.tile([CL, QL], f32)  # [32, 1024] = 2 banks
        for kp in range(2):
            for qh in range(2):
                qs = slice(512 * qh, 512 * (qh + 1))
                nc.tensor.matmul(
                    s_ps[:, qs],
                    W8[:, kp],
                    q8[:, kp, :, qs],
                    start=(kp == 0),
                    stop=False,
                    perf_mode=DR,
                )
        sq(4, nc.vector)
        sq(7, nc.vector)
        sq(6, nc.gpsimd)
        sq(5, nc.vector)
        dummy_mm(3)

        # ---------------- prototypes: d-half 1 --------------------------
        for jp in range(SJP):
            nc.tensor.matmul(
                p_ps[:, 512:1024],
                oh[:],
                s8[:, 1, jp],
                start=(jp == 0),
                stop=(jp == SJP - 1),
                perf_mode=DR,
            )

        # ||q||^2 chunk-pairs 0, 1 while the W chain B evacuates
        for kp in (0, 1):
            for qh in range(2):
                qs = slice(512 * qh, 512 * (qh + 1))
                nc.tensor.matmul(
                    s_ps[:, qs], on8[:], qsq[:, kp, :, qs],
                    start=False, stop=False, perf_mode=DR,
                )

        # W chain B
        psbB = sb.tile([CL, 512], bf16)
        w_chain(1, psbB)

        # tail matmuls: ||q||^2 pair 2 first (no W dependency), then the
        # half-1 gram, then pair 3 closes the banks
        for qh in range(2):
            qs = slice(512 * qh, 512 * (qh + 1))
            nc.tensor.matmul(
                s_ps[:, qs], on8[:], qsq[:, 2, :, qs],
                start=False, stop=False, perf_mode=DR,
            )
        for kp in range(2, 4):
            for qh in range(2):
                qs = slice(512 * qh, 512 * (qh + 1))
                nc.tensor.matmul(
                    s_ps[:, qs],
                    W8[:, kp],
                    q8[:, kp, :, qs],
                    start=False,
                    stop=False,
                    perf_mode=DR,
                )
        for qh in range(2):
            qs = slice(512 * qh, 512 * (qh + 1))
            nc.tensor.matmul(
                s_ps[:, qs], on8[:], qsq[:, 3, :, qs],
                start=False, stop=True, perf_mode=DR,
            )

        # ||p||^2 half B + total
        sq_dumpB = sb.tile([CL, 512], bf16)
        pnB = sb.tile([CL, 1], f32)
        pn = sb.tile([CL, 1], f32)
        nc.scalar.activation(sq_dumpB[:], psbB[:], AF.Square, accum_out=pnB[:])
        nc.vector.tensor_add(pn[:], pnA[:], pnB[:])

        # ------- sqrt(+||p||^2), negate, store (2 q-halves pipelined) ---
        for qh in range(2):
            qs = slice(512 * qh, 512 * (qh + 1))
            lt = sb.tile([CL, 512], f32, name=f"lt{qh}")
            ltn = sb.tile([CL, 512], f32, name=f"ltn{qh}")
            nc.scalar.activation(lt[:], s_ps[:, qs], AF.Sqrt, bias=pn[:, 0:1])
            nc.vector.tensor_scalar_mul(ltn[:], lt[:], -1.0)
            nc.sync.dma_start(out[:, qs], ltn[:])



def _build():
    if "nc" in _CACHE:
        return _CACHE["nc"]
    from concourse import bacc, mybir, tile

    f32 = mybir.dt.float32
    fp8 = mybir.dt.float8e4
    nc = bacc.Bacc(
        "TRN2",
        target_bir_lowering=False,
        debug=False,
        enable_asserts=False,
        num_devices=NCORES,
    )
    sup = nc.dram_tensor("sup", [128, 2 * SJP * 2 * 512], fp8, kind="ExternalInput").ap()
    qt = nc.dram_tensor("qt", [128, KP * 2 * QL], fp8, kind="ExternalInput").ap()
    oh_in = nc.dram_tensor("oh", [128, 2 * CL], fp8, kind="ExternalInput").ap()
    out = nc.dram_tensor("out", [CL, QL], f32, kind="ExternalOutput").ap()
    with tile.TileContext(nc) as tc:
        _emit(nc, tc, sup, qt, oh_in, out)
    nc.compile()
    _CACHE["nc"] = nc
    return nc


def _shard(embeddings):
    import ml_dtypes

    emb = np.asarray(embeddings, dtype=np.float32).reshape(C, 2 * S, D)
    support = emb[:, :S, :]                       # [64, 64, 1024]
    queries = emb[:, S:, :].reshape(C * S, D)     # [4096, 1024]

    # one-hot: oh[p, o, c] = 1 iff c == p//4 (same for every chunk-pair)
    p = np.arange(128)[:, None, None]
    c = np.arange(CL)[None, None, :]
    oh = (c == p // 4) + np.zeros((1, 2, 1), dtype=bool)
    oh = np.ascontiguousarray(
        oh.astype(ml_dtypes.float8_e4m3).reshape(128, 2 * CL)
    )

    sups = []
    for j in range(CB):
        # [32, 64, 1024] -> [c, jp, o, m, h, 512] -> [c, m, h, jp, o, 512]
        sj = support[CL * j : CL * (j + 1)].reshape(CL, SJP, 2, 4, 2, 512)
        sj = sj.transpose(0, 3, 4, 1, 2, 5).reshape(128, 2 * SJP * 2 * 512)
        sups.append(np.ascontiguousarray(sj.astype(ml_dtypes.float8_e4m3)))
    qts = []
    for i in range(QB):
        # [1024q, 1024d] -> T -> [4kp, 2o, 128p, 1024q] -> [p, kp, o, q]
        qi = queries[QL * i : QL * (i + 1)].T.reshape(KP, 2, 128, QL)
        qi = qi.transpose(2, 0, 1, 3).reshape(128, KP * 2 * QL)
        qts.append(np.ascontiguousarray(qi.astype(ml_dtypes.float8_e4m3)))

    in_maps = []
    for k in range(NCORES):
        i, j = k // CB, k % CB
        in_maps.append({"sup": sups[j], "qt": qts[i], "oh": oh})
    return in_maps


def _gather(outs):
    """outs: list of 8 per-core [CL, QL] arrays -> full [C*S, C] f32."""
    logits = np.empty((C * S, C), dtype=np.float32)
    for k in range(NCORES):
        i, j = k // CB, k % CB
        logits[QL * i : QL * (i + 1), CL * j : CL * (j + 1)] = np.asarray(
            outs[k], dtype=np.float32
        ).T
    return logits


def kernel(embeddings_stacked, n_classes, n_support, **_unused):
    assert int(n_classes) == C and int(n_support) == S
    emb = np.asarray(embeddings_stacked)
    assert emb.shape == (C * 2 * S, D), emb.shape

    from concourse import bass_utils

    nc = _build()
    in_maps = _shard(emb)
    try:
        res = bass_utils.run_bass_kernel_spmd(
            nc, in_maps, core_ids=list(range(NCORES))
        )
    except Exception:
        # transient device/runtime hiccups have been observed; retry once
        res = bass_utils.run_bass_kernel_spmd(
            nc, in_maps, core_ids=list(range(NCORES))
        )
    return _gather([res.results[k]["out"] for k in range(NCORES)])


if __name__ == "__main__":
    rng = np.random.default_rng(0)
    emb = rng.standard_normal((C * 2 * S, D), dtype=np.float32)
    got = kernel(emb, C, S)
    print("kernel output", got.shape, got.dtype)


# revision 10
# speedup vs baseline: 1.0321x; 1.0028x over previous
"""Trainium2 Bass kernel for BaselineProtonet (retrieval_knn).

logits[q, c] = -||query_q - proto_c||_2
  proto_c = mean of 64 support embeddings of class c
  embeddings_stacked: [64 classes * (64 support + 64 query), 1024] f32

Sharding (8 cores, 2D grid): 4 query blocks x 2 class blocks. Core
(i, j) owns query rows 1024*i..1024*(i+1) and classes 32*j..32*(j+1).
Per-core wire traffic: 2MB fp8 support + 2MB bf16 queries (queries must
be bf16: the ||q||^2 squares are elementwise on DVE, and fp8 runs at 1x
(1.2-3.5us per [128,1024] chunk, run-to-run unstable) while bf16 gets
the 2x packed mode (~0.85us). DMA streams at ~340GB/s in 512KB slices
(4KB/partition lines -- the SDMA engines are descriptor-latency bound,
so smaller lines halve throughput).

Host-side shard prep (layout/encoding only, no arithmetic): support
swizzled d-half-major with partition p owning class p//4 in every
256-row DoubleRow chunk; queries feature-major bf16.

Stream order and overlap (all compute tracks DMA completion sems, which
trail the last byte of a slice by ~2us):
  supH0 | qryA (d-half0) | supH1 | qryB (d-half1)
  - protos h0 (fp8 DoubleRow one-hot matmuls) right after warmup;
    evac (1/64) -> PE transpose -> ACT *-2 gives W16 for d-half0, so
    gram+||q||^2 matmuls for chunks 0-3 run mid-kernel
  - protos h1 + W chain B overlap the qryB stream; only chunk 4-7
    gram/ones matmuls, sqrt(+||p||^2), negate, store trail the last byte
  - dummy no-dep matmuls keep the PE HAM clock-gate warm across waits
"""

import numpy as np

C = 64          # classes
S = 64          # support per class (== queries per class)
D = 1024        # embedding dim
NCORES = 8
QB = 4          # query blocks
CB = 2          # class blocks
CL = C // CB    # 32 classes per core
QL = (C * S) // QB          # 1024 query rows per core
SJP = (CL * S) // 256       # 8 support chunk-pairs per core
DCH = D // 128              # 8 feature chunks

_CACHE = {}


def _emit(nc, tc, sup, qt, oh_in, out):
    """Emit the per-core tile program.

    sup:   [128, 2*SJP*2*512] fp8 DRAM (support, [h, jp, o, d512] cols,
                                        partition p owns class p//4)
    qt:    [128, DCH*QL] bf16 DRAM     (queries, feature-major)
    oh_in: [128, 2*CL] fp8 DRAM        (DoubleRow one-hot class mask)
    out:   [CL, QL] f32 DRAM           (negated distances, class-major)
    """
    from concourse import masks, mybir

    f32 = mybir.dt.float32
    bf16 = mybir.dt.bfloat16
    fp8 = mybir.dt.float8e4
    AF = mybir.ActivationFunctionType
    DR = mybir.MatmulPerfMode.DoubleRow

    with (
        tc.tile_pool(name="sb", bufs=1) as sb,
        tc.tile_pool(name="ps", bufs=1, space="PSUM") as ps,
    ):
        # warm the PE clock first-thing: HAM un-throttles only after
        # ~3.4us of sustained matmul activity
        wm_in = sb.tile([128, 512], bf16)
        nc.gpsimd.memset(wm_in[:], 0.0)
        wm_ps = ps.tile([128, 512], f32)
        for _ in range(8):
            nc.tensor.matmul(
                wm_ps[:], wm_in[:, 0:128], wm_in[:], start=True, stop=True
            )

        def dummy_mm(n=1):
            # no-dep matmuls: scheduled only when nothing real is ready,
            # they bridge DMA/evac waits so the HAM gate stays warm
            for _ in range(n):
                nc.tensor.matmul(
                    wm_ps[:], wm_in[:, 0:128], wm_in[:], start=True, stop=True
                )

        # ---------------- input DMA ring (one sync-queue FIFO) ----------
        oh = sb.tile([128, 2, CL], fp8)
        nc.scalar.dma_start(
            oh[:], oh_in[:, :].rearrange("p (o c) -> p o c", o=2)
        )
        q16 = sb.tile([128, DCH, QL], bf16)
        s8 = sb.tile([128, 2, SJP, 2, 512], fp8)

        def q_slice(g):  # 512KB: d-chunks 2g, 2g+1, all queries
            nc.sync.dma_start(
                q16[:, 2 * g : 2 * (g + 1)],
                qt[:, 2 * QL * g : 2 * QL * (g + 1)].rearrange(
                    "p (k q) -> p k q", k=2
                ),
            )

        def s_slice(h, jp0, njp):  # d-half h, chunk-pairs jp0..jp0+njp
            nc.sync.dma_start(
                s8[:, h, jp0 : jp0 + njp],
                sup[
                    :, 8192 * h + 1024 * jp0 : 8192 * h + 1024 * (jp0 + njp)
                ].rearrange("p (jp o d) -> p jp o d", jp=njp, o=2),
            )

        s_slice(0, 0, 4)    # supH0a: protos start right after warmup
        s_slice(0, 4, 4)    # supH0b
        q_slice(0)          # qryA1: d-chunks 0,1
        q_slice(1)          # qryA2: d-chunks 2,3
        s_slice(1, 0, 4)    # supH1a
        s_slice(1, 4, 4)    # supH1b
        q_slice(2)          # qryB1: d-chunks 4,5
        q_slice(3)          # qryB2: d-chunks 6,7

        # ---------------- constants -------------------------------------
        ident = sb.tile([128, 128], bf16)
        masks.make_identity(nc, ident[:])
        ones16 = sb.tile([128, CL], bf16)
        nc.gpsimd.memset(ones16[:], 1.0)

        # preload the sqrt ACT table off the critical path
        warm_sq = sb.tile([1, 1], f32)
        nc.gpsimd.memset(warm_sq[:], 1.0)
        nc.scalar.activation(warm_sq[:], warm_sq[:], AF.Sqrt)

        # ---------------- prototypes: d-half 0 --------------------------
        p_ps = ps.tile([CL, D], f32)  # [32, 1024] = 2 banks
        for jp in range(SJP):
            nc.tensor.matmul(
                p_ps[:, 0:512],
                oh[:],
                s8[:, 0, jp],
                start=(jp == 0),
                stop=(jp == SJP - 1),
                perf_mode=DR,
            )

        dummy_mm(4)

        # W chain: evac protos (1/64) to bf16, transpose per 128-d chunk,
        # scale by -2; quarter [32,256] evacs pipeline the chain
        psbA = sb.tile([CL, 512], bf16)
        psbB = sb.tile([CL, 512], bf16)
        pt_ps = ps.tile([128, DCH, CL], bf16)  # [128, 8*32] = 1 bank
        W16 = sb.tile([128, DCH, CL], bf16)

        def w_chain(h, psb):
            src_ps = p_ps[:, 512 * h : 512 * (h + 1)]
            for u in range(2):
                us = slice(256 * u, 256 * (u + 1))
                nc.scalar.mul(psb[:, us], src_ps[:, us], 1.0 / S)
                for kk in range(2):
                    k = 4 * h + 2 * u + kk
                    nc.tensor.transpose(
                        pt_ps[:, k],
                        psb[:, 256 * u + 128 * kk : 256 * u + 128 * (kk + 1)],
                        ident[0:CL, 0:CL],
                    )
                k0 = 4 * h + 2 * u
                nc.scalar.mul(
                    W16[:, k0 : k0 + 2].rearrange("p k c -> p (k c)"),
                    pt_ps[:, k0 : k0 + 2].rearrange("p k c -> p (k c)"),
                    -2.0,
                )

        w_chain(0, psbA)

        # ||q||^2 squares on DVE only (bf16 2x mode, ~0.85us per chunk)
        qsq = sb.tile([128, DCH, QL], bf16)

        def sq(k):
            nc.vector.tensor_mul(qsq[:, k], q16[:, k], q16[:, k])

        for k in range(4):
            sq(k)

        # gram + ||q||^2 matmuls for d-half0 run mid-kernel
        s_ps = ps.tile([CL, QL], f32)  # [32, 1024] = 2 banks

        def gram(k, start=False):
            for qh in range(2):
                qs = slice(512 * qh, 512 * (qh + 1))
                nc.tensor.matmul(
                    s_ps[:, qs], W16[:, k], q16[:, k, qs],
                    start=start, stop=False,
                )

        def ones(k, stop=False):
            for qh in range(2):
                qs = slice(512 * qh, 512 * (qh + 1))
                nc.tensor.matmul(
                    s_ps[:, qs], ones16[:], qsq[:, k, qs],
                    start=False, stop=(stop and qh == 1),
                )

        gram(0, start=True)
        ones(0)
        gram(1)
        ones(1)
        gram(2)
        ones(2)
        gram(3)
        ones(3)

        # ||p||^2 half A on ACT (idle window), accumulate along d
        sq_dumpA = sb.tile([CL, 512], bf16)
        pnA = sb.tile([CL, 1], f32)
        nc.scalar.activation(sq_dumpA[:], psbA[:], AF.Square, accum_out=pnA[:])

        # ---------------- prototypes: d-half 1 --------------------------
        for jp in range(SJP):
            nc.tensor.matmul(
                p_ps[:, 512:1024],
                oh[:],
                s8[:, 1, jp],
                start=(jp == 0),
                stop=(jp == SJP - 1),
                perf_mode=DR,
            )
        dummy_mm(3)

        w_chain(1, psbB)

        for k in range(4, 8):
            sq(k)

        # tail matmuls for d-half1
        gram(4)
        ones(4)
        gram(5)
        ones(5)
        gram(6)
        ones(6)
        gram(7)
        ones(7, stop=True)

        # ||p||^2 half B + total
        sq_dumpB = sb.tile([CL, 512], bf16)
        pnB = sb.tile([CL, 1], f32)
        pn = sb.tile([CL, 1], f32)
        nc.scalar.activation(sq_dumpB[:], psbB[:], AF.Square, accum_out=pnB[:])
        nc.vector.tensor_add(pn[:], pnA[:], pnB[:])

        # ------- sqrt(+||p||^2), negate, store (2 q-halves pipelined) ---
        for qh in range(2):
            qs = slice(512 * qh, 512 * (qh + 1))
            lt = sb.tile([CL, 512], f32, name=f"lt{qh}")
            ltn = sb.tile([CL, 512], f32, name=f"ltn{qh}")
            nc.scalar.activation(lt[:], s_ps[:, qs], AF.Sqrt, bias=pn[:, 0:1])
            nc.vector.tensor_scalar_mul(ltn[:], lt[:], -1.0)
            nc.sync.dma_start(out[:, qs], ltn[:])


def _build():
    if "nc" in _CACHE:
        return _CACHE["nc"]
    from concourse import bacc, mybir, tile

    f32 = mybir.dt.float32
    bf16 = mybir.dt.bfloat16
    fp8 = mybir.dt.float8e4
    nc = bacc.Bacc(
        "TRN2",
        target_bir_lowering=False,
        debug=False,
        enable_asserts=False,
        num_devices=NCORES,
    )
    sup = nc.dram_tensor("sup", [128, 2 * SJP * 2 * 512], fp8, kind="ExternalInput").ap()
    qt = nc.dram_tensor("qt", [128, DCH * QL], bf16, kind="ExternalInput").ap()
    oh_in = nc.dram_tensor("oh", [128, 2 * CL], fp8, kind="ExternalInput").ap()
    out = nc.dram_tensor("out", [CL, QL], f32, kind="ExternalOutput").ap()
    with tile.TileContext(nc) as tc:
        _emit(nc, tc, sup, qt, oh_in, out)
    nc.compile()
    _CACHE["nc"] = nc
    return nc


def _shard(embeddings):
    import ml_dtypes

    emb = np.asarray(embeddings, dtype=np.float32).reshape(C, 2 * S, D)
    support = emb[:, :S, :]                       # [64, 64, 1024]
    queries = emb[:, S:, :].reshape(C * S, D)     # [4096, 1024]

    # one-hot: oh[p, o, c] = 1 iff c == p//4 (same for every chunk-pair)
    p = np.arange(128)[:, None, None]
    c = np.arange(CL)[None, None, :]
    oh = (c == p // 4) + np.zeros((1, 2, 1), dtype=bool)
    oh = np.ascontiguousarray(
        oh.astype(ml_dtypes.float8_e4m3).reshape(128, 2 * CL)
    )

    sups = []
    for j in range(CB):
        # [32, 64, 1024] -> [c, jp, o, m, h, 512] -> [c, m, h, jp, o, 512]
        sj = support[CL * j : CL * (j + 1)].reshape(CL, SJP, 2, 4, 2, 512)
        sj = sj.transpose(0, 3, 4, 1, 2, 5).reshape(128, 2 * SJP * 2 * 512)
        sups.append(np.ascontiguousarray(sj.astype(ml_dtypes.float8_e4m3)))
    qts = []
    for i in range(QB):
        # [1024q, 1024d] -> T -> [8k, 128p, 1024q] -> [p, k, q]
        qi = queries[QL * i : QL * (i + 1)].T.reshape(DCH, 128, QL)
        qi = qi.transpose(1, 0, 2).reshape(128, DCH * QL)
        qts.append(np.ascontiguousarray(qi.astype(ml_dtypes.bfloat16)))

    in_maps = []
    for k in range(NCORES):
        i, j = k // CB, k % CB
        in_maps.append({"sup": sups[j], "qt": qts[i], "oh": oh})
    return in_maps


def _gather(outs):
    """outs: list of 8 per-core [CL, QL] arrays -> full [C*S, C] f32."""
    logits = np.empty((C * S, C), dtype=np.float32)
    for k in range(NCORES):
        i, j = k // CB, k % CB
        logits[QL * i : QL * (i + 1), CL * j : CL * (j + 1)] = np.asarray(
            outs[k], dtype=np.float32
        ).T
    return logits


def kernel(embeddings_stacked, n_classes, n_support, **_unused):
    assert int(n_classes) == C and int(n_support) == S
    emb = np.asarray(embeddings_stacked)
    assert emb.shape == (C * 2 * S, D), emb.shape

    from concourse import bass_utils

    nc = _build()
    in_maps = _shard(emb)
    try:
        res = bass_utils.run_bass_kernel_spmd(
            nc, in_maps, core_ids=list(range(NCORES))
        )
    except Exception:
        # transient device/runtime hiccups have been observed; retry once
        res = bass_utils.run_bass_kernel_spmd(
            nc, in_maps, core_ids=list(range(NCORES))
        )
    return _gather([res.results[k]["out"] for k in range(NCORES)])


if __name__ == "__main__":
    rng = np.random.default_rng(0)
    emb = rng.standard_normal((C * 2 * S, D), dtype=np.float32)
    got = kernel(emb, C, S)
    print("kernel output", got.shape, got.dtype)
